# revision 5
# baseline (speedup 1.0000x reference)
"""Bass/Tile TRN2 kernel for LowRankMixtureCrossNet (B=16384, N=1024, L=3, E=8, R=64).

Strategy:
- Data-parallel: batch sharded 8 ways (2048 tokens/core), weights replicated.
- On-chip layout is feature-major (x^T): SBUF tiles [128 feat, T=512 tokens].
  Host pre-transposes x and pre-packs the weights.
- All matmul operands in bf16 (1 cycle/column PE streaming; measured
  233 ns/MM for back-to-back 512-col bf16 matmuls incl. weight loads).
  PSUM accumulation stays fp32. Simulated numerics: 7.5e-3 max-rel-err
  (gate 2e-2); measured on HW 6.4e-3. fp8 variants simulate at 2.2-3e-2
  and fp8-weight matmuls measured SLOWER (313 ns/MM mixed-dtype penalty),
  so bf16 everywhere is both the precision and the speed floor.
- TWO token tiles (A, B) are processed as an interleaved wavefront per
  layer: A's serial softmax/elemwise chain (ACT/DVE) runs while B's
  matmul bursts occupy the PE, and vice versa. The residual updates
  (t2/xcur) are emitted AFTER both tiles' u-projection bursts so the DVE
  queue never blocks the other tile's in-chain ops.
- Per layer, per token tile:
    logits[e,t]  = sum_n gate_w[e,n] x[n,t]           (8 chunk matmuls, M=8)
    gates        = softmax over e: exp (ACT), partition sum + broadcast via
                   tiny PE matmuls against ones vectors, reciprocal+mult (DVE)
    v            = 4 expert-pair matmuls x 8 K-chunks, M=128 (2 experts x R=64)
    rvg          = relu(v) * gates  (gates folded in early:
                   g*U@relu(C@(g*relu(v))) == g*u since g>0 commutes via relu)
    cg           = relu(Cblk @ rvg)        (block-diag 2-expert C matmuls)
    w            = Uall^T.T @ cg           (8 n-chunks x 4 K-pair matmuls)
    xnew[n,t]    = (w[n,t] + bias[n]) * x0[n,t] + x[n,t]
      (softmax makes sum_e g = 1, so bias needs no gate weighting)
  The final layer's xnew is written fp32 and DMA'd out on the Pool queue.
"""
import numpy as np
from contextlib import ExitStack

import concourse.bass as bass
import concourse.tile as tile
from concourse import bacc, mybir
from concourse.bass_utils import run_bass_kernel_spmd

B, N, L, E, R = 16384, 1024, 3, 8, 64
NCORES = 8
BC = B // NCORES      # tokens per core
T = 512               # token tile (matmul free dim)
NT = BC // T          # token tiles per core
NPAIR = NT // 2       # interleaved tile pairs
NCH = N // 128        # feature chunks
NP = E // 2           # expert pairs
ER = E * R            # 512

f32 = mybir.dt.float32
bf16 = mybir.dt.bfloat16
AFT = mybir.ActivationFunctionType
ALU = mybir.AluOpType


def build(niter: int = 1, dma_in_loop=True, elemwise=True, pool_xcur=True,
          psum=(2, 2, 2, 1), hp=False):
    MDT = bf16
    nc = bacc.Bacc(trn_type="TRN2", debug=False, num_devices=NCORES)

    xT_d = nc.dram_tensor("xT", [N, BC], MDT, kind="ExternalInput")
    vt_d = nc.dram_tensor("VT", [L, N, ER], MDT, kind="ExternalInput")
    ut_d = nc.dram_tensor("UT", [L, ER, N], MDT, kind="ExternalInput")
    cb_d = nc.dram_tensor("CB", [L, 128, NP * 128], MDT, kind="ExternalInput")
    gt_d = nc.dram_tensor("GT", [N, E], MDT, kind="ExternalInput")
    sel_d = nc.dram_tensor("SEL", [E, NP * 128], MDT, kind="ExternalInput")
    bs_d = nc.dram_tensor("BS", [128, L * NCH], f32, kind="ExternalInput")
    on8_d = nc.dram_tensor("ON8", [E, 1], MDT, kind="ExternalInput")
    on1_d = nc.dram_tensor("ON1", [1, E], MDT, kind="ExternalInput")
    y_d = nc.dram_tensor("y", [N, BC], f32, kind="ExternalOutput")

    with tile.TileContext(nc) as tc, ExitStack() as ctx:
        wp = ctx.enter_context(tc.tile_pool(name="wp", bufs=1))
        xp = ctx.enter_context(tc.tile_pool(name="xp", bufs=2))
        xc = ctx.enter_context(tc.tile_pool(name="xc", bufs=2))
        yp = ctx.enter_context(tc.tile_pool(name="yp", bufs=1))
        wk = ctx.enter_context(tc.tile_pool(name="wk", bufs=3))
        g8 = ctx.enter_context(tc.tile_pool(name="g8", bufs=2))
        pv = ctx.enter_context(tc.tile_pool(name="pv", bufs=psum[0], space="PSUM"))
        pcg = ctx.enter_context(tc.tile_pool(name="pcg", bufs=psum[1], space="PSUM"))
        pw = ctx.enter_context(tc.tile_pool(name="pw", bufs=psum[2], space="PSUM"))
        psm = ctx.enter_context(tc.tile_pool(name="psm", bufs=psum[3], space="PSUM"))

        # ---- persistent weights ----
        vt, ut, cbt, gt = {}, {}, {}, {}

        def load_layer_weights(l, eng):
            tv = wp.tile([128, NCH * ER], MDT, tag=f"vt{l}", name=f"vt{l}")
            eng.dma_start(tv[:].rearrange("p (c e) -> p c e", c=NCH),
                          vt_d[l].rearrange("(c p) e -> p c e", p=128))
            for c in range(NCH):
                vt[l, c] = tv[:, c * ER:(c + 1) * ER]
            tu = wp.tile([128, NP * N], MDT, tag=f"ut{l}", name=f"ut{l}")
            eng.dma_start(tu[:].rearrange("p (k n) -> p k n", k=NP),
                          ut_d[l].rearrange("(k p) n -> p k n", p=128))
            for k in range(NP):
                ut[l, k] = tu[:, k * N:(k + 1) * N]
            t = wp.tile([128, NP * 128], MDT, tag=f"cb{l}", name=f"cb{l}")
            eng.dma_start(t[:], cb_d[l, :, :])
            cbt[l] = t

        # tiny operands + layer-0 V on the sync queue (critical path to the
        # first matmuls); the bulk (U0 + layers 1-2) on the scalar queue,
        # which is idle during preload.
        gtt = wp.tile([128, NCH * E], MDT, tag="gt", name="gtt")
        nc.sync.dma_start(gtt[:].rearrange("p (c e) -> p c e", c=NCH),
                          gt_d[:, :].rearrange("(c p) e -> p c e", p=128))
        for c in range(NCH):
            gt[c] = gtt[:, c * E:(c + 1) * E]
        selt = wp.tile([E, NP * 128], MDT, tag="sel", name="selt")
        nc.sync.dma_start(selt[:], sel_d[:, :])
        bst = wp.tile([128, L * NCH], f32, tag="bs", name="bst")
        nc.sync.dma_start(bst[:], bs_d[:, :])
        on8 = wp.tile([E, 1], MDT, tag="on8", name="on8")
        nc.sync.dma_start(on8[:], on8_d[:, :])
        on1 = wp.tile([1, E], MDT, tag="on1", name="on1")
        nc.sync.dma_start(on1[:], on1_d[:, :])
        tv = wp.tile([128, NCH * ER], MDT, tag="vt0", name="vt0")
        nc.sync.dma_start(tv[:].rearrange("p (c e) -> p c e", c=NCH),
                          vt_d[0].rearrange("(c p) e -> p c e", p=128))
        for c in range(NCH):
            vt[0, c] = tv[:, c * ER:(c + 1) * ER]
        tu = wp.tile([128, NP * N], MDT, tag="ut0", name="ut0")
        nc.scalar.dma_start(tu[:].rearrange("p (k n) -> p k n", k=NP),
                            ut_d[0].rearrange("(k p) n -> p k n", p=128))
        for k in range(NP):
            ut[0, k] = tu[:, k * N:(k + 1) * N]
        t0cb = wp.tile([128, NP * 128], MDT, tag="cb0", name="cb0")
        nc.scalar.dma_start(t0cb[:], cb_d[0, :, :])
        cbt[0] = t0cb
        for l in range(1, L):
            load_layer_weights(l, nc.scalar)

        uid = [0]
        x0_shared = {}

        class TileState:
            pass

        def start_tile(t, s):
            """Allocate per-tile tiles and start x0 DMA."""
            u = uid[0]
            uid[0] += 1
            s.u = u
            s.t = t
            s.ab = "ab"[t % 2]
            ab = s.ab
            if dma_in_loop or t not in x0_shared:
                x0t = [xp.tile([128, T], MDT, tag=f"x0{ab}_{c}", name=f"x0_{u}_{c}")
                       for c in range(NCH)]
                for c in range(NCH):
                    nc.sync.dma_start(x0t[c][:],
                                      xT_d[c * 128:(c + 1) * 128, t * T:(t + 1) * T])
                s.x0 = [x0t[c][:] for c in range(NCH)]
                if not dma_in_loop:
                    x0_shared[t] = s.x0
            else:
                s.x0 = x0_shared[t]
            xcurt = [xc.tile([128, T], MDT, tag=f"xc{ab}_{c}", name=f"xc_{u}_{c}")
                     for c in range(NCH)]
            s.xcur = [xcurt[c][:] for c in range(NCH)]
            s.yt = [yp.tile([128, T], f32, tag=f"y{ab}_{c}", name=f"y_{u}_{c}")
                    for c in range(NCH)]

        def emit_gates_v(s, l):
            """PE burst 1: gate logits + v matmuls; ACT: exp + relu trail."""
            u, ab = s.u, s.ab
            xin = s.x0 if l == 0 else s.xcur
            s.xin = xin
            lg = psm.tile([E, T], f32, tag=f"lg{ab}", name=f"lg_{u}_{l}")
            for c in range(NCH):
                nc.tensor.matmul(lg[:], lhsT=gt[c][:], rhs=xin[c],
                                 start=(c == 0), stop=(c == NCH - 1))
            if elemwise:
                eh = g8.tile([E, T], MDT, tag=f"eh{ab}", name=f"eh_{u}_{l}")
                nc.scalar.activation(eh[:], lg[:], AFT.Exp)
                s.eh = eh
            s.rvs = {}
            for p in range(NP):
                vp = pv.tile([128, T], f32, tag="v", name=f"v_{u}_{l}_{p}")
                for c in range(NCH):
                    nc.tensor.matmul(vp[:], lhsT=vt[l, c][:, p * 128:(p + 1) * 128],
                                     rhs=xin[c],
                                     start=(c == 0), stop=(c == NCH - 1))
                if elemwise:
                    rv = wk.tile([128, T], MDT, tag=f"rv{ab}", name=f"rv_{u}_{l}_{p}",
                                 bufs=4)
                    nc.scalar.activation(rv[:], vp[:], AFT.Relu)
                    s.rvs[p] = rv

        def emit_S(s, l):
            if not elemwise:
                return
            S = psm.tile([1, T], f32, tag=f"lg{s.ab}", name=f"S_{s.u}_{l}")
            nc.tensor.matmul(S[:], lhsT=on8[:], rhs=s.eh[:], start=True, stop=True)
            s.S = S

        def emit_recip(s, l):
            if not elemwise:
                return
            r1 = g8.tile([1, T], MDT, tag=f"r1{s.ab}", name=f"r1_{s.u}_{l}")
            with nc.allow_low_precision(reason="softmax recip to bf16"):
                nc.vector.reciprocal(r1[:], s.S[:])
            s.r1 = r1

        def emit_gn(s, l):
            if not elemwise:
                s.gn = selt
                return
            r8 = psm.tile([E, T], f32, tag=f"lg{s.ab}", name=f"r8_{s.u}_{l}")
            nc.tensor.matmul(r8[:], lhsT=on1[:], rhs=s.r1[:], start=True, stop=True)
            gn = g8.tile([E, T], MDT, tag=f"gn{s.ab}", name=f"gn_{s.u}_{l}")
            nc.vector.tensor_tensor(gn[:], s.eh[:], r8[:], op=ALU.mult)
            s.gn = gn

        def emit_cu(s, l):
            """PE burst 2: gate broadcast, C matmuls, u-projection."""
            u, ab = s.u, s.ab
            g2s = {}
            for p in range(NP):
                g2 = pcg.tile([128, T], f32, tag="cg2", name=f"g2_{u}_{l}_{p}")
                nc.tensor.matmul(g2[:], lhsT=selt[:, p * 128:(p + 1) * 128],
                                 rhs=s.gn[:, 0:T], start=True, stop=True)
                g2s[p] = g2
            rvgs = {}
            for p in range(NP):
                if elemwise:
                    rvg = wk.tile([128, T], MDT, tag=f"rvg{ab}", name=f"rvg_{u}_{l}_{p}",
                                  bufs=4)
                    nc.vector.tensor_tensor(rvg[:], s.rvs[p][:], g2s[p][:], op=ALU.mult)
                    rvgs[p] = rvg[:]
                else:
                    rvgs[p] = s.x0[p]
            cg = {}
            for p in range(NP):
                cp = pcg.tile([128, T], f32, tag="cg2", name=f"c_{u}_{l}_{p}")
                nc.tensor.matmul(cp[:], lhsT=cbt[l][:, p * 128:(p + 1) * 128],
                                 rhs=rvgs[p], start=True, stop=True)
                if elemwise:
                    cgp = wk.tile([128, T], MDT, tag=f"cg{ab}{p}", name=f"cg_{u}_{l}_{p}",
                                  bufs=1)
                    nc.scalar.activation(cgp[:], cp[:], AFT.Relu)
                    cg[p] = cgp[:]
                else:
                    cg[p] = s.x0[p]
            s.wms = []
            for m in range(NCH):
                wm = pw.tile([128, T], f32, tag="w", name=f"w_{u}_{l}_{m}")
                for k in range(NP):
                    nc.tensor.matmul(wm[:], lhsT=ut[l, k][:, m * 128:(m + 1) * 128],
                                     rhs=cg[k],
                                     start=(k == 0), stop=(k == NP - 1))
                s.wms.append(wm)

        def emit_resid(s, l):
            """Residual update: xnew = (w + bias) * x0 + xin."""
            u, ab = s.u, s.ab
            last = l == L - 1
            xout = [s.yt[c][:] for c in range(NCH)] if last else s.xcur
            for m in range(NCH):
                if elemwise:
                    t2 = wk.tile([128, T], MDT, tag=f"t2{ab}", name=f"t2_{u}_{l}_{m}",
                                 bufs=4)
                    ops = []
                    ops.append(lambda m=m, t2=t2: nc.vector.scalar_tensor_tensor(
                        t2[:], s.wms[m][:], bst[:, l * NCH + m:l * NCH + m + 1],
                        s.x0[m], op0=ALU.add, op1=ALU.mult))
                    eng = nc.gpsimd if pool_xcur else nc.vector
                    ops.append(lambda m=m, t2=t2, eng=eng: eng.tensor_tensor(
                        xout[m], t2[:], s.xin[m], op=ALU.add))
                    if hp:
                        with tc.high_priority():
                            for op in ops:
                                op()
                    else:
                        for op in ops:
                            op()
                else:
                    nc.vector.tensor_copy(xout[m], s.x0[m])
            if last and (dma_in_loop or s.t < 2):
                for c in range(NCH):
                    nc.gpsimd.dma_start(
                        y_d[c * 128:(c + 1) * 128, s.t * T:(s.t + 1) * T],
                        s.yt[c][:])

        def pair_wave(tp):
            A, Bs = TileState(), TileState()
            start_tile(2 * tp, A)
            start_tile(2 * tp + 1, Bs)
            for l in range(L):
                emit_gates_v(A, l)
                emit_gates_v(Bs, l)
                emit_S(A, l)
                emit_S(Bs, l)
                emit_recip(A, l)
                emit_recip(Bs, l)
                emit_gn(A, l)
                emit_gn(Bs, l)
                emit_cu(A, l)
                emit_cu(Bs, l)
                emit_resid(A, l)
                emit_resid(Bs, l)

        if niter == 1:
            for tp in range(NPAIR):
                pair_wave(tp)
        else:
            with tc.For_i(0, niter, 1) as _:
                for tp in range(NPAIR):
                    pair_wave(tp)

    nc.compile()
    return nc


def pack_inputs(x, U, V, C, bias, gate_w):
    """Host-side packing into the DRAM layouts the kernel expects."""
    import ml_dtypes
    x = np.asarray(x, dtype=np.float32)
    U = np.asarray(U, dtype=np.float32)
    V = np.asarray(V, dtype=np.float32)
    C = np.asarray(C, dtype=np.float32)
    bias = np.asarray(bias, dtype=np.float32)
    gate_w = np.asarray(gate_w, dtype=np.float32)

    xT = np.ascontiguousarray(x.T)                          # [N, B]
    VT = np.ascontiguousarray(V.transpose(0, 3, 1, 2).reshape(L, N, ER))
    UT = np.ascontiguousarray(U.transpose(0, 1, 3, 2).reshape(L, ER, N))
    CB = np.zeros((L, 128, NP * 128), np.float32)
    for l in range(L):
        for p in range(NP):
            CB[l, 0:64, p * 128:p * 128 + 64] = C[l, 2 * p].T
            CB[l, 64:128, p * 128 + 64:p * 128 + 128] = C[l, 2 * p + 1].T
    GT = np.ascontiguousarray(gate_w.T)                     # [N, E]
    SEL = np.zeros((E, NP * 128), np.float32)
    for p in range(NP):
        SEL[2 * p, p * 128:p * 128 + 64] = 1.0
        SEL[2 * p + 1, p * 128 + 64:p * 128 + 128] = 1.0
    BS = np.zeros((128, L * NCH), np.float32)
    for l in range(L):
        for m in range(NCH):
            BS[:, l * NCH + m] = bias[l, m * 128:(m + 1) * 128]

    ON8 = np.ones((E, 1), np.float32)
    ON1 = np.ones((1, E), np.float32)
    shared = {"VT": VT, "UT": UT, "CB": CB, "GT": GT, "SEL": SEL, "BS": BS,
              "ON8": ON8, "ON1": ON1}
    for k in ("VT", "UT", "CB", "GT", "SEL", "ON8", "ON1"):
        shared[k] = shared[k].astype(ml_dtypes.bfloat16)
    xT = xT.astype(ml_dtypes.bfloat16)
    in_maps = []
    for i in range(NCORES):
        m = dict(shared)
        m["xT"] = np.ascontiguousarray(xT[:, i * BC:(i + 1) * BC])
        in_maps.append(m)
    return in_maps


def run(nc, in_maps):
    res = run_bass_kernel_spmd(nc, in_maps, core_ids=list(range(NCORES)))
    yT = np.empty((N, B), np.float32)
    for i in range(NCORES):
        yT[:, i * BC:(i + 1) * BC] = res.results[i]["y"]
    return np.ascontiguousarray(yT.T)


_NC_CACHE = {}


def kernel(x, U, V, C, bias, gate_w):
    x = np.asarray(x)
    assert x.shape == (B, N), f"expected x {(B, N)}, got {x.shape}"
    if "nc" not in _NC_CACHE:
        _NC_CACHE["nc"] = build(niter=1)
    in_maps = pack_inputs(x, U, V, C, bias, gate_w)
    return run(_NC_CACHE["nc"], in_maps)


# revision 8
# speedup vs baseline: 1.2652x; 1.2652x over previous
"""Bass/Tile TRN2 kernel for LowRankMixtureCrossNet (B=16384, N=1024, L=3, E=8, R=64).

Strategy:
- Data-parallel: batch sharded 8 ways (2048 tokens/core), weights replicated.
- On-chip layout is feature-major (x^T): SBUF tiles [128 feat, T=512 tokens].
  Host pre-transposes x and pre-packs the weights.
- All matmul operands in bf16 (1 cycle/column PE streaming; measured
  233 ns/MM for back-to-back 512-col bf16 matmuls incl. weight loads).
  PSUM accumulation stays fp32. Simulated numerics: 7.5e-3 max-rel-err
  (gate 2e-2); measured on HW 6.4e-3. fp8 variants simulate at 2.2-3e-2
  and fp8-weight matmuls measured SLOWER (313 ns/MM mixed-dtype penalty),
  so bf16 everywhere is both the precision and the speed floor.
- TWO token tiles (A, B) are processed as an interleaved wavefront per
  layer: A's serial softmax/elemwise chain (ACT/DVE) runs while B's
  matmul bursts occupy the PE, and vice versa. The residual updates
  (t2/xcur) are emitted AFTER both tiles' u-projection bursts so the DVE
  queue never blocks the other tile's in-chain ops.
- Per layer, per token tile:
    logits[e,t]  = sum_n gate_w[e,n] x[n,t]           (8 chunk matmuls, M=8)
    gates        = softmax over e: exp (ACT), partition sum + broadcast via
                   tiny PE matmuls against ones vectors, reciprocal+mult (DVE)
    v            = 4 expert-pair matmuls x 8 K-chunks, M=128 (2 experts x R=64)
    rvg          = relu(v) * gates  (gates folded in early:
                   g*U@relu(C@(g*relu(v))) == g*u since g>0 commutes via relu)
    cg           = relu(Cblk @ rvg)        (block-diag 2-expert C matmuls)
    w            = Uall^T.T @ cg           (8 n-chunks x 4 K-pair matmuls)
    xnew[n,t]    = (w[n,t] + bias[n]) * x0[n,t] + x[n,t]
      (softmax makes sum_e g = 1, so bias needs no gate weighting)
  The final layer's xnew is written fp32 and DMA'd out on the Pool queue.
"""
import numpy as np
from contextlib import ExitStack

import concourse.bass as bass
import concourse.tile as tile
from concourse import bacc, mybir
from concourse.bass_utils import run_bass_kernel_spmd

B, N, L, E, R = 16384, 1024, 3, 8, 64
NCORES = 8
BC = B // NCORES      # tokens per core
T = 512               # token tile (matmul free dim)
NT = BC // T          # token tiles per core
NPAIR = NT // 2       # interleaved tile pairs
NCH = N // 128        # feature chunks
NP = E // 2           # expert pairs
ER = E * R            # 512

f32 = mybir.dt.float32
bf16 = mybir.dt.bfloat16
AFT = mybir.ActivationFunctionType
ALU = mybir.AluOpType


def build(niter: int = 1, dma_in_loop=True, elemwise=True, pool_xcur=True,
          psum=(2, 2, 2, 1), hp=False):
    MDT = bf16
    nc = bacc.Bacc(trn_type="TRN2", debug=False, num_devices=NCORES)

    xT_d = nc.dram_tensor("xT", [N, BC], MDT, kind="ExternalInput")
    vt_d = nc.dram_tensor("VT", [L, N, ER], MDT, kind="ExternalInput")
    ut_d = nc.dram_tensor("UT", [L, ER, N], MDT, kind="ExternalInput")
    cb_d = nc.dram_tensor("CB", [L, 128, NP * 128], MDT, kind="ExternalInput")
    gt_d = nc.dram_tensor("GT", [N, E], MDT, kind="ExternalInput")
    sel_d = nc.dram_tensor("SEL", [E, NP * 128], MDT, kind="ExternalInput")
    bs_d = nc.dram_tensor("BS", [128, L * NCH], f32, kind="ExternalInput")
    on8_d = nc.dram_tensor("ON8", [E, 1], MDT, kind="ExternalInput")
    on1_d = nc.dram_tensor("ON1", [1, E], MDT, kind="ExternalInput")
    y_d = nc.dram_tensor("y", [N, BC], f32, kind="ExternalOutput")

    with tile.TileContext(nc) as tc, ExitStack() as ctx:
        wp = ctx.enter_context(tc.tile_pool(name="wp", bufs=1))
        xp = ctx.enter_context(tc.tile_pool(name="xp", bufs=2))
        xc = ctx.enter_context(tc.tile_pool(name="xc", bufs=2))
        yp = ctx.enter_context(tc.tile_pool(name="yp", bufs=1))
        wk = ctx.enter_context(tc.tile_pool(name="wk", bufs=3))
        g8 = ctx.enter_context(tc.tile_pool(name="g8", bufs=2))
        pv = ctx.enter_context(tc.tile_pool(name="pv", bufs=psum[0], space="PSUM"))
        pcg = ctx.enter_context(tc.tile_pool(name="pcg", bufs=psum[1], space="PSUM"))
        pw = ctx.enter_context(tc.tile_pool(name="pw", bufs=psum[2], space="PSUM"))
        psm = ctx.enter_context(tc.tile_pool(name="psm", bufs=psum[3], space="PSUM"))

        # ---- persistent weights ----
        vt, ut, cbt, gt = {}, {}, {}, {}

        def load_layer_weights(l, eng):
            tv = wp.tile([128, NCH * ER], MDT, tag=f"vt{l}", name=f"vt{l}")
            eng.dma_start(tv[:].rearrange("p (c e) -> p c e", c=NCH),
                          vt_d[l].rearrange("(c p) e -> p c e", p=128))
            for c in range(NCH):
                vt[l, c] = tv[:, c * ER:(c + 1) * ER]
            tu = wp.tile([128, NP * N], MDT, tag=f"ut{l}", name=f"ut{l}")
            eng.dma_start(tu[:].rearrange("p (k n) -> p k n", k=NP),
                          ut_d[l].rearrange("(k p) n -> p k n", p=128))
            for k in range(NP):
                ut[l, k] = tu[:, k * N:(k + 1) * N]
            t = wp.tile([128, NP * 128], MDT, tag=f"cb{l}", name=f"cb{l}")
            eng.dma_start(t[:], cb_d[l, :, :])
            cbt[l] = t

        # tiny operands + layer-0 V on the sync queue (critical path to the
        # first matmuls); the bulk (U0 + layers 1-2) on the scalar queue,
        # which is idle during preload.
        gtt = wp.tile([128, NCH * E], MDT, tag="gt", name="gtt")
        nc.sync.dma_start(gtt[:].rearrange("p (c e) -> p c e", c=NCH),
                          gt_d[:, :].rearrange("(c p) e -> p c e", p=128))
        for c in range(NCH):
            gt[c] = gtt[:, c * E:(c + 1) * E]
        selt = wp.tile([E, NP * 128], MDT, tag="sel", name="selt")
        nc.sync.dma_start(selt[:], sel_d[:, :])
        bst = wp.tile([128, L * NCH], f32, tag="bs", name="bst")
        nc.sync.dma_start(bst[:], bs_d[:, :])
        on8 = wp.tile([E, 1], MDT, tag="on8", name="on8")
        nc.sync.dma_start(on8[:], on8_d[:, :])
        on1 = wp.tile([1, E], MDT, tag="on1", name="on1")
        nc.sync.dma_start(on1[:], on1_d[:, :])
        tv = wp.tile([128, NCH * ER], MDT, tag="vt0", name="vt0")
        nc.sync.dma_start(tv[:].rearrange("p (c e) -> p c e", c=NCH),
                          vt_d[0].rearrange("(c p) e -> p c e", p=128))
        for c in range(NCH):
            vt[0, c] = tv[:, c * ER:(c + 1) * ER]
        tu = wp.tile([128, NP * N], MDT, tag="ut0", name="ut0")
        nc.scalar.dma_start(tu[:].rearrange("p (k n) -> p k n", k=NP),
                            ut_d[0].rearrange("(k p) n -> p k n", p=128))
        for k in range(NP):
            ut[0, k] = tu[:, k * N:(k + 1) * N]
        t0cb = wp.tile([128, NP * 128], MDT, tag="cb0", name="cb0")
        nc.scalar.dma_start(t0cb[:], cb_d[0, :, :])
        cbt[0] = t0cb
        for l in range(1, L):
            load_layer_weights(l, nc.scalar)

        uid = [0]
        x0_shared = {}

        class TileState:
            pass

        def start_tile(t, s):
            """Allocate per-tile tiles and start x0 DMA."""
            u = uid[0]
            uid[0] += 1
            s.u = u
            s.t = t
            s.ab = "ab"[t % 2]
            ab = s.ab
            if dma_in_loop or t not in x0_shared:
                x0t = [xp.tile([128, T], MDT, tag=f"x0{ab}_{c}", name=f"x0_{u}_{c}")
                       for c in range(NCH)]
                for c in range(NCH):
                    nc.sync.dma_start(x0t[c][:],
                                      xT_d[c * 128:(c + 1) * 128, t * T:(t + 1) * T])
                s.x0 = [x0t[c][:] for c in range(NCH)]
                if not dma_in_loop:
                    x0_shared[t] = s.x0
            else:
                s.x0 = x0_shared[t]
            xcurt = [xc.tile([128, T], MDT, tag=f"xc{ab}_{c}", name=f"xc_{u}_{c}")
                     for c in range(NCH)]
            s.xcur = [xcurt[c][:] for c in range(NCH)]
            s.yt = [yp.tile([128, T], f32, tag=f"y{ab}_{c}", name=f"y_{u}_{c}")
                    for c in range(NCH)]

        def emit_burst(s, l):
            """PE burst for one tile: gate logits + v matmuls with the
            softmax's tiny PE ops (S, r8, g2) threaded between v p-groups so
            the gate broadcast is ready by burst end without stalling PE.
            ACT: exp + relus trail; DVE: recip, gn, rvg trail."""
            u, ab = s.u, s.ab
            xin = s.x0 if l == 0 else s.xcur
            s.xin = xin
            lg = psm.tile([E, T], f32, tag=f"lg{ab}", name=f"lg_{u}_{l}")
            for c in range(NCH):
                nc.tensor.matmul(lg[:], lhsT=gt[c][:], rhs=xin[c],
                                 start=(c == 0), stop=(c == NCH - 1))
            if elemwise:
                eh = g8.tile([E, T], MDT, tag=f"eh{ab}", name=f"eh_{u}_{l}")
                nc.scalar.activation(eh[:], lg[:], AFT.Exp)
                s.eh = eh
            s.rvs = {}
            g2s = {}

            def vgroup(p):
                vp = pv.tile([128, T], f32, tag="v", name=f"v_{u}_{l}_{p}")
                for c in range(NCH):
                    nc.tensor.matmul(vp[:], lhsT=vt[l, c][:, p * 128:(p + 1) * 128],
                                     rhs=xin[c],
                                     start=(c == 0), stop=(c == NCH - 1))
                if elemwise:
                    rv = wk.tile([128, T], MDT, tag=f"rv{ab}", name=f"rv_{u}_{l}_{p}",
                                 bufs=4)
                    nc.scalar.activation(rv[:], vp[:], AFT.Relu)
                    s.rvs[p] = rv

            vgroup(0)
            vgroup(1)
            if elemwise:
                S = psm.tile([1, T], f32, tag=f"lg{ab}", name=f"S_{u}_{l}")
                nc.tensor.matmul(S[:], lhsT=on8[:], rhs=s.eh[:], start=True, stop=True)
                r1 = g8.tile([1, T], MDT, tag=f"r1{ab}", name=f"r1_{u}_{l}")
                with nc.allow_low_precision(reason="softmax recip to bf16"):
                    nc.vector.reciprocal(r1[:], S[:])
            vgroup(2)
            if elemwise:
                r8 = psm.tile([E, T], f32, tag=f"lg{ab}", name=f"r8_{u}_{l}")
                nc.tensor.matmul(r8[:], lhsT=on1[:], rhs=r1[:], start=True, stop=True)
                gn = g8.tile([E, T], MDT, tag=f"gn{ab}", name=f"gn_{u}_{l}")
                nc.vector.tensor_tensor(gn[:], s.eh[:], r8[:], op=ALU.mult)
            else:
                gn = selt
            vgroup(3)
            for p in range(NP):
                g2 = pcg.tile([128, T], f32, tag="cg2", name=f"g2_{u}_{l}_{p}")
                nc.tensor.matmul(g2[:], lhsT=selt[:, p * 128:(p + 1) * 128],
                                 rhs=gn[:, 0:T], start=True, stop=True)
                g2s[p] = g2
            s.rvgs = {}
            for p in range(NP):
                if elemwise:
                    rvg = wk.tile([128, T], MDT, tag=f"rvg{ab}", name=f"rvg_{u}_{l}_{p}",
                                  bufs=4)
                    nc.vector.tensor_tensor(rvg[:], s.rvs[p][:], g2s[p][:], op=ALU.mult)
                    s.rvgs[p] = rvg[:]
                else:
                    s.rvgs[p] = s.x0[p]

        def emit_cu(s, l):
            """PE: C matmuls + u-projection; ACT: cg relus trail."""
            u, ab = s.u, s.ab
            cg = {}
            for p in range(NP):
                cp = pcg.tile([128, T], f32, tag="cg2", name=f"c_{u}_{l}_{p}")
                nc.tensor.matmul(cp[:], lhsT=cbt[l][:, p * 128:(p + 1) * 128],
                                 rhs=s.rvgs[p], start=True, stop=True)
                if elemwise:
                    cgp = wk.tile([128, T], MDT, tag=f"cg{ab}{p}", name=f"cg_{u}_{l}_{p}",
                                  bufs=1)
                    nc.scalar.activation(cgp[:], cp[:], AFT.Relu)
                    cg[p] = cgp[:]
                else:
                    cg[p] = s.x0[p]
            s.wms = []
            for m in range(NCH):
                wm = pw.tile([128, T], f32, tag="w", name=f"w_{u}_{l}_{m}")
                for k in range(NP):
                    nc.tensor.matmul(wm[:], lhsT=ut[l, k][:, m * 128:(m + 1) * 128],
                                     rhs=cg[k],
                                     start=(k == 0), stop=(k == NP - 1))
                s.wms.append(wm)

        def emit_resid(s, l):
            """Residual update: xnew = (w + bias) * x0 + xin. The last
            layer's add (off the critical chain) goes to the Pool engine."""
            u, ab = s.u, s.ab
            last = l == L - 1
            xout = [s.yt[c][:] for c in range(NCH)] if last else s.xcur
            for m in range(NCH):
                if elemwise:
                    t2 = wk.tile([128, T], MDT, tag=f"t2{ab}", name=f"t2_{u}_{l}_{m}",
                                 bufs=4)
                    ops = []
                    ops.append(lambda m=m, t2=t2: nc.vector.scalar_tensor_tensor(
                        t2[:], s.wms[m][:], bst[:, l * NCH + m:l * NCH + m + 1],
                        s.x0[m], op0=ALU.add, op1=ALU.mult))
                    eng = nc.gpsimd if (pool_xcur and last) else nc.vector
                    ops.append(lambda m=m, t2=t2, eng=eng: eng.tensor_tensor(
                        xout[m], t2[:], s.xin[m], op=ALU.add))
                    if hp:
                        with tc.high_priority():
                            for op in ops:
                                op()
                    else:
                        for op in ops:
                            op()
                else:
                    nc.vector.tensor_copy(xout[m], s.x0[m])
            if last and (dma_in_loop or s.t < 2):
                for c in range(NCH):
                    nc.gpsimd.dma_start(
                        y_d[c * 128:(c + 1) * 128, s.t * T:(s.t + 1) * T],
                        s.yt[c][:])

        def pair_wave(tp):
            A, Bs = TileState(), TileState()
            start_tile(2 * tp, A)
            start_tile(2 * tp + 1, Bs)
            for l in range(L):
                emit_burst(A, l)
                emit_burst(Bs, l)
                emit_cu(A, l)
                emit_cu(Bs, l)
                emit_resid(A, l)
                emit_resid(Bs, l)

        if niter == 1:
            for tp in range(NPAIR):
                pair_wave(tp)
        else:
            with tc.For_i(0, niter, 1) as _:
                for tp in range(NPAIR):
                    pair_wave(tp)

    nc.compile()
    return nc


def pack_inputs(x, U, V, C, bias, gate_w):
    """Host-side packing into the DRAM layouts the kernel expects."""
    import ml_dtypes
    x = np.asarray(x, dtype=np.float32)
    U = np.asarray(U, dtype=np.float32)
    V = np.asarray(V, dtype=np.float32)
    C = np.asarray(C, dtype=np.float32)
    bias = np.asarray(bias, dtype=np.float32)
    gate_w = np.asarray(gate_w, dtype=np.float32)

    xT = np.ascontiguousarray(x.T)                          # [N, B]
    VT = np.ascontiguousarray(V.transpose(0, 3, 1, 2).reshape(L, N, ER))
    UT = np.ascontiguousarray(U.transpose(0, 1, 3, 2).reshape(L, ER, N))
    CB = np.zeros((L, 128, NP * 128), np.float32)
    for l in range(L):
        for p in range(NP):
            CB[l, 0:64, p * 128:p * 128 + 64] = C[l, 2 * p].T
            CB[l, 64:128, p * 128 + 64:p * 128 + 128] = C[l, 2 * p + 1].T
    GT = np.ascontiguousarray(gate_w.T)                     # [N, E]
    SEL = np.zeros((E, NP * 128), np.float32)
    for p in range(NP):
        SEL[2 * p, p * 128:p * 128 + 64] = 1.0
        SEL[2 * p + 1, p * 128 + 64:p * 128 + 128] = 1.0
    BS = np.zeros((128, L * NCH), np.float32)
    for l in range(L):
        for m in range(NCH):
            BS[:, l * NCH + m] = bias[l, m * 128:(m + 1) * 128]

    ON8 = np.ones((E, 1), np.float32)
    ON1 = np.ones((1, E), np.float32)
    shared = {"VT": VT, "UT": UT, "CB": CB, "GT": GT, "SEL": SEL, "BS": BS,
              "ON8": ON8, "ON1": ON1}
    for k in ("VT", "UT", "CB", "GT", "SEL", "ON8", "ON1"):
        shared[k] = shared[k].astype(ml_dtypes.bfloat16)
    xT = xT.astype(ml_dtypes.bfloat16)
    in_maps = []
    for i in range(NCORES):
        m = dict(shared)
        m["xT"] = np.ascontiguousarray(xT[:, i * BC:(i + 1) * BC])
        in_maps.append(m)
    return in_maps


def run(nc, in_maps):
    res = run_bass_kernel_spmd(nc, in_maps, core_ids=list(range(NCORES)))
    yT = np.empty((N, B), np.float32)
    for i in range(NCORES):
        yT[:, i * BC:(i + 1) * BC] = res.results[i]["y"]
    return np.ascontiguousarray(yT.T)


_NC_CACHE = {}


def kernel(x, U, V, C, bias, gate_w):
    x = np.asarray(x)
    assert x.shape == (B, N), f"expected x {(B, N)}, got {x.shape}"
    if "nc" not in _NC_CACHE:
        _NC_CACHE["nc"] = build(niter=1)
    in_maps = pack_inputs(x, U, V, C, bias, gate_w)
    return run(_NC_CACHE["nc"], in_maps)


# revision 12
# speedup vs baseline: 1.3917x; 1.1000x over previous
"""Bass/Tile TRN2 kernel for LowRankMixtureCrossNet (B=16384, N=1024, L=3, E=8, R=64).

Strategy:
- Data-parallel: batch sharded 8 ways (2048 tokens/core), weights replicated.
- On-chip layout is feature-major (x^T): SBUF tiles [128 feat, T=512 tokens].
  Host pre-transposes x and pre-packs the weights.
- All matmul operands in bf16 (1 cycle/column PE streaming; measured
  233 ns/MM for back-to-back 512-col bf16 matmuls incl. weight loads).
  PSUM accumulation stays fp32. Simulated numerics: 7.5e-3 max-rel-err
  (gate 2e-2); measured on HW 6.4e-3. fp8 variants simulate at 2.2-3e-2
  and fp8-weight matmuls measured SLOWER (313 ns/MM mixed-dtype penalty),
  so bf16 everywhere is both the precision and the speed floor.
- TWO token tiles (A, B) are processed as an interleaved wavefront per
  layer: A's serial softmax/elemwise chain (ACT/DVE) runs while B's
  matmul bursts occupy the PE, and vice versa. The residual updates
  (t2/xcur) are emitted AFTER both tiles' u-projection bursts so the DVE
  queue never blocks the other tile's in-chain ops.
- Per layer, per token tile:
    logits[e,t]  = sum_n gate_w[e,n] x[n,t]           (8 chunk matmuls, M=8)
    gates        = softmax over e: exp (ACT), partition sum + broadcast via
                   tiny PE matmuls against ones vectors, reciprocal+mult (DVE)
    v            = 4 expert-pair matmuls x 8 K-chunks, M=128 (2 experts x R=64)
    rvg          = relu(v) * gates  (gates folded in early:
                   g*U@relu(C@(g*relu(v))) == g*u since g>0 commutes via relu)
    cg           = relu(Cblk @ rvg)        (block-diag 2-expert C matmuls)
    w            = Uall^T.T @ cg           (8 n-chunks x 4 K-pair matmuls)
    xnew[n,t]    = (w[n,t] + bias[n]) * x0[n,t] + x[n,t]
      (softmax makes sum_e g = 1, so bias needs no gate weighting)
  The final layer's xnew is written fp32 and DMA'd out on the Pool queue.
"""
import numpy as np
from contextlib import ExitStack

import concourse.bass as bass
import concourse.tile as tile
from concourse import bacc, mybir
from concourse.bass_utils import run_bass_kernel_spmd

B, N, L, E, R = 16384, 1024, 3, 8, 64
NCORES = 8
BC = B // NCORES      # tokens per core
T = 512               # token tile (matmul free dim)
NT = BC // T          # token tiles per core
NPAIR = NT // 2       # interleaved tile pairs
NCH = N // 128        # feature chunks
NP = E // 2           # expert pairs
ER = E * R            # 512

f32 = mybir.dt.float32
bf16 = mybir.dt.bfloat16
AFT = mybir.ActivationFunctionType
ALU = mybir.AluOpType


def build(niter: int = 1, dma_in_loop=True, elemwise=True, pool_xcur=True,
          psum=(2, 2, 2, 1), hp=False):
    MDT = bf16
    nc = bacc.Bacc(trn_type="TRN2", debug=False, num_devices=NCORES)

    xT_d = nc.dram_tensor("xT", [N, BC], MDT, kind="ExternalInput")
    vt_d = nc.dram_tensor("VT", [L, N, ER], MDT, kind="ExternalInput")
    ut_d = nc.dram_tensor("UT", [L, ER, N], MDT, kind="ExternalInput")
    cb_d = nc.dram_tensor("CB", [L, 128, NP * 128], MDT, kind="ExternalInput")
    gt_d = nc.dram_tensor("GT", [N, E], MDT, kind="ExternalInput")
    sel_d = nc.dram_tensor("SEL", [E, NP * 128], MDT, kind="ExternalInput")
    bs_d = nc.dram_tensor("BS", [128, L * NCH], f32, kind="ExternalInput")
    on8_d = nc.dram_tensor("ON8", [E, 1], MDT, kind="ExternalInput")
    on1_d = nc.dram_tensor("ON1", [1, E], MDT, kind="ExternalInput")
    y_d = nc.dram_tensor("y", [N, BC], f32, kind="ExternalOutput")

    with tile.TileContext(nc) as tc, ExitStack() as ctx:
        wp = ctx.enter_context(tc.tile_pool(name="wp", bufs=1))
        xp = ctx.enter_context(tc.tile_pool(name="xp", bufs=2))
        xc = ctx.enter_context(tc.tile_pool(name="xc", bufs=2))
        yp = ctx.enter_context(tc.tile_pool(name="yp", bufs=1))
        wk = ctx.enter_context(tc.tile_pool(name="wk", bufs=3))
        g8 = ctx.enter_context(tc.tile_pool(name="g8", bufs=1))
        pv = ctx.enter_context(tc.tile_pool(name="pv", bufs=psum[0], space="PSUM"))
        pcg = ctx.enter_context(tc.tile_pool(name="pcg", bufs=psum[1], space="PSUM"))
        pw = ctx.enter_context(tc.tile_pool(name="pw", bufs=psum[2], space="PSUM"))
        psm = ctx.enter_context(tc.tile_pool(name="psm", bufs=psum[3], space="PSUM"))

        # ---- persistent weights ----
        vt, ut, cbt, gt = {}, {}, {}, {}

        def load_layer_weights(l, eng):
            tv = wp.tile([128, NCH * ER], MDT, tag=f"vt{l}", name=f"vt{l}")
            eng.dma_start(tv[:].rearrange("p (c e) -> p c e", c=NCH),
                          vt_d[l].rearrange("(c p) e -> p c e", p=128))
            for c in range(NCH):
                vt[l, c] = tv[:, c * ER:(c + 1) * ER]
            tu = wp.tile([128, NP * N], MDT, tag=f"ut{l}", name=f"ut{l}")
            eng.dma_start(tu[:].rearrange("p (k n) -> p k n", k=NP),
                          ut_d[l].rearrange("(k p) n -> p k n", p=128))
            for k in range(NP):
                ut[l, k] = tu[:, k * N:(k + 1) * N]
            t = wp.tile([128, NP * 128], MDT, tag=f"cb{l}", name=f"cb{l}")
            eng.dma_start(t[:], cb_d[l, :, :])
            cbt[l] = t

        # tiny operands + layer-0 V on the sync queue (critical path to the
        # first matmuls); the bulk (U0 + layers 1-2) on the scalar queue,
        # which is idle during preload.
        gtt = wp.tile([128, NCH * E], MDT, tag="gt", name="gtt")
        nc.sync.dma_start(gtt[:].rearrange("p (c e) -> p c e", c=NCH),
                          gt_d[:, :].rearrange("(c p) e -> p c e", p=128))
        for c in range(NCH):
            gt[c] = gtt[:, c * E:(c + 1) * E]
        selt = wp.tile([E, NP * 128], MDT, tag="sel", name="selt")
        nc.sync.dma_start(selt[:], sel_d[:, :])
        bst = wp.tile([128, L * NCH], f32, tag="bs", name="bst")
        nc.sync.dma_start(bst[:], bs_d[:, :])
        on8 = wp.tile([E, 1], MDT, tag="on8", name="on8")
        nc.sync.dma_start(on8[:], on8_d[:, :])
        on1 = wp.tile([1, E], MDT, tag="on1", name="on1")
        nc.sync.dma_start(on1[:], on1_d[:, :])
        tv = wp.tile([128, NCH * ER], MDT, tag="vt0", name="vt0")
        nc.sync.dma_start(tv[:].rearrange("p (c e) -> p c e", c=NCH),
                          vt_d[0].rearrange("(c p) e -> p c e", p=128))
        for c in range(NCH):
            vt[0, c] = tv[:, c * ER:(c + 1) * ER]
        tu = wp.tile([128, NP * N], MDT, tag="ut0", name="ut0")
        nc.scalar.dma_start(tu[:].rearrange("p (k n) -> p k n", k=NP),
                            ut_d[0].rearrange("(k p) n -> p k n", p=128))
        for k in range(NP):
            ut[0, k] = tu[:, k * N:(k + 1) * N]
        t0cb = wp.tile([128, NP * 128], MDT, tag="cb0", name="cb0")
        nc.scalar.dma_start(t0cb[:], cb_d[0, :, :])
        cbt[0] = t0cb
        for l in range(1, L):
            load_layer_weights(l, nc.scalar)

        uid = [0]
        x0_shared = {}

        class TileState:
            pass

        def start_tile(t, s):
            """Allocate per-tile wide tiles and start the x0 DMA (one wide
            transfer; chunk c occupies columns [c*T, (c+1)*T))."""
            u = uid[0]
            uid[0] += 1
            s.u = u
            s.t = t
            s.ab = "ab"[t % 2]
            ab = s.ab
            if dma_in_loop or t not in x0_shared:
                x0w = xp.tile([128, NCH * T], MDT, tag=f"x0{ab}", name=f"x0_{u}")
                nc.sync.dma_start(
                    x0w[:].rearrange("p (c t) -> p c t", c=NCH),
                    xT_d.rearrange("(c p) b -> p c b", p=128)[:, :, t * T:(t + 1) * T])
                s.x0w = x0w
                s.x0 = [x0w[:, c * T:(c + 1) * T] for c in range(NCH)]
                if not dma_in_loop:
                    x0_shared[t] = (s.x0w, s.x0)
            else:
                s.x0w, s.x0 = x0_shared[t]
            xcw = xc.tile([128, NCH * T], MDT, tag=f"xc{ab}", name=f"xc_{u}")
            s.xcw = xcw
            s.xcur = [xcw[:, c * T:(c + 1) * T] for c in range(NCH)]
            s.ytw = yp.tile([128, NCH * T], f32, tag=f"y{ab}", name=f"y_{u}")
            s.yt = [s.ytw[:, c * T:(c + 1) * T] for c in range(NCH)]

        def emit_burst(s, l):
            """PE burst for one tile: gate logits + v matmuls with the
            softmax's tiny PE ops (S, r8, g2) threaded between v p-groups so
            the gate broadcast is ready by burst end without stalling PE.
            ACT: exp + relus trail; DVE: recip, gn, rvg trail."""
            u, ab = s.u, s.ab
            xin = s.x0 if l == 0 else s.xcur
            s.xin = xin
            s.xinw = s.x0w if l == 0 else s.xcw
            lg = psm.tile([E, T], f32, tag=f"lg{ab}", name=f"lg_{u}_{l}")
            for c in range(NCH):
                nc.tensor.matmul(lg[:], lhsT=gt[c][:], rhs=xin[c],
                                 start=(c == 0), stop=(c == NCH - 1))
            if elemwise:
                eh = g8.tile([E, T], MDT, tag=f"eh{ab}", name=f"eh_{u}_{l}")
                nc.scalar.activation(eh[:], lg[:], AFT.Exp)
                s.eh = eh
            s.rvs = {}
            g2s = {}

            def vgroup(p):
                vp = pv.tile([128, T], f32, tag="v", name=f"v_{u}_{l}_{p}")
                for c in range(NCH):
                    nc.tensor.matmul(vp[:], lhsT=vt[l, c][:, p * 128:(p + 1) * 128],
                                     rhs=xin[c],
                                     start=(c == 0), stop=(c == NCH - 1))
                if elemwise:
                    rv = wk.tile([128, T], MDT, tag=f"rv{ab}", name=f"rv_{u}_{l}_{p}",
                                 bufs=4)
                    nc.scalar.activation(rv[:], vp[:], AFT.Relu)
                    s.rvs[p] = rv

            vgroup(0)
            vgroup(1)
            if elemwise:
                S = psm.tile([1, T], f32, tag=f"lg{ab}", name=f"S_{u}_{l}")
                nc.tensor.matmul(S[:], lhsT=on8[:], rhs=s.eh[:], start=True, stop=True)
                r1 = g8.tile([1, T], MDT, tag=f"r1{ab}", name=f"r1_{u}_{l}")
                with nc.allow_low_precision(reason="softmax recip to bf16"):
                    nc.vector.reciprocal(r1[:], S[:])
            vgroup(2)
            if elemwise:
                r8 = psm.tile([E, T], f32, tag=f"lg{ab}", name=f"r8_{u}_{l}")
                nc.tensor.matmul(r8[:], lhsT=on1[:], rhs=r1[:], start=True, stop=True)
                gn = g8.tile([E, T], MDT, tag=f"gn{ab}", name=f"gn_{u}_{l}")
                nc.vector.tensor_tensor(gn[:], s.eh[:], r8[:], op=ALU.mult)
            else:
                gn = selt
            vgroup(3)
            for p in range(NP):
                g2 = pcg.tile([128, T], f32, tag="cg2", name=f"g2_{u}_{l}_{p}")
                nc.tensor.matmul(g2[:], lhsT=selt[:, p * 128:(p + 1) * 128],
                                 rhs=gn[:, 0:T], start=True, stop=True)
                g2s[p] = g2
            s.rvgs = {}
            for p in range(NP):
                if elemwise:
                    rvg = wk.tile([128, T], MDT, tag=f"rvg{ab}", name=f"rvg_{u}_{l}_{p}",
                                  bufs=4)
                    nc.vector.tensor_tensor(rvg[:], s.rvs[p][:], g2s[p][:], op=ALU.mult)
                    s.rvgs[p] = rvg[:]
                else:
                    s.rvgs[p] = s.x0[p]

        def emit_cu(s, l):
            """PE: C matmuls + u-projection; ACT: cg relus trail."""
            u, ab = s.u, s.ab
            cg = {}
            for p in range(NP):
                cp = pcg.tile([128, T], f32, tag="cg2", name=f"c_{u}_{l}_{p}")
                nc.tensor.matmul(cp[:], lhsT=cbt[l][:, p * 128:(p + 1) * 128],
                                 rhs=s.rvgs[p], start=True, stop=True)
                if elemwise:
                    cgp = wk.tile([128, T], MDT, tag=f"cg{ab}{p}", name=f"cg_{u}_{l}_{p}",
                                  bufs=1)
                    nc.scalar.activation(cgp[:], cp[:], AFT.Relu)
                    cg[p] = cgp[:]
                else:
                    cg[p] = s.x0[p]
            s.wms = []
            for m in range(NCH):
                wm = pw.tile([128, T], f32, tag="w", name=f"w_{u}_{l}_{m}")
                for k in range(NP):
                    nc.tensor.matmul(wm[:], lhsT=ut[l, k][:, m * 128:(m + 1) * 128],
                                     rhs=cg[k],
                                     start=(k == 0), stop=(k == NP - 1))
                s.wms.append(wm)

        def emit_resid(s, l):
            """Residual update: xnew = (w + bias) * x0 + xin. t2 slices are
            written per-chunk (PSUM-bound), then the residual add runs as two
            half-wide ops; the last layer's adds (off the critical chain) go
            to the Pool engine and the output leaves as one wide DMA."""
            u, ab = s.u, s.ab
            last = l == L - 1
            outw = s.ytw if last else s.xcw
            if not elemwise:
                nc.vector.tensor_copy(outw[:], s.x0w[:])
            else:
                t2w = wk.tile([128, NCH * T], MDT, tag=f"t2{ab}", name=f"t2_{u}_{l}",
                              bufs=1)
                ops = []
                for m in range(NCH):
                    ops.append(lambda m=m: nc.vector.scalar_tensor_tensor(
                        t2w[:, m * T:(m + 1) * T], s.wms[m][:],
                        bst[:, l * NCH + m:l * NCH + m + 1],
                        s.x0[m], op0=ALU.add, op1=ALU.mult))
                eng = nc.gpsimd if (pool_xcur and last) else nc.vector
                H = NCH * T // 2
                for h in range(2):
                    ops.append(lambda h=h, eng=eng: eng.tensor_tensor(
                        outw[:, h * H:(h + 1) * H], t2w[:, h * H:(h + 1) * H],
                        s.xinw[:, h * H:(h + 1) * H], op=ALU.add))
                if hp:
                    with tc.high_priority():
                        for op in ops:
                            op()
                else:
                    for op in ops:
                        op()
            if last and (dma_in_loop or s.t < 2):
                nc.gpsimd.dma_start(
                    y_d.rearrange("(c p) b -> p c b", p=128)[:, :, s.t * T:(s.t + 1) * T],
                    s.ytw[:].rearrange("p (c t) -> p c t", c=NCH))

        def pair_wave(tp):
            A, Bs = TileState(), TileState()
            start_tile(2 * tp, A)
            start_tile(2 * tp + 1, Bs)
            for l in range(L):
                emit_burst(A, l)
                emit_burst(Bs, l)
                emit_cu(A, l)
                emit_cu(Bs, l)
                emit_resid(A, l)
                emit_resid(Bs, l)

        if niter == 1:
            for tp in range(NPAIR):
                pair_wave(tp)
        else:
            with tc.For_i(0, niter, 1) as _:
                for tp in range(NPAIR):
                    pair_wave(tp)

    nc.compile()
    return nc


def pack_inputs(x, U, V, C, bias, gate_w):
    """Host-side packing into the DRAM layouts the kernel expects."""
    import ml_dtypes
    x = np.asarray(x, dtype=np.float32)
    U = np.asarray(U, dtype=np.float32)
    V = np.asarray(V, dtype=np.float32)
    C = np.asarray(C, dtype=np.float32)
    bias = np.asarray(bias, dtype=np.float32)
    gate_w = np.asarray(gate_w, dtype=np.float32)

    xT = np.ascontiguousarray(x.T)                          # [N, B]
    VT = np.ascontiguousarray(V.transpose(0, 3, 1, 2).reshape(L, N, ER))
    UT = np.ascontiguousarray(U.transpose(0, 1, 3, 2).reshape(L, ER, N))
    CB = np.zeros((L, 128, NP * 128), np.float32)
    for l in range(L):
        for p in range(NP):
            CB[l, 0:64, p * 128:p * 128 + 64] = C[l, 2 * p].T
            CB[l, 64:128, p * 128 + 64:p * 128 + 128] = C[l, 2 * p + 1].T
    GT = np.ascontiguousarray(gate_w.T)                     # [N, E]
    SEL = np.zeros((E, NP * 128), np.float32)
    for p in range(NP):
        SEL[2 * p, p * 128:p * 128 + 64] = 1.0
        SEL[2 * p + 1, p * 128 + 64:p * 128 + 128] = 1.0
    BS = np.zeros((128, L * NCH), np.float32)
    for l in range(L):
        for m in range(NCH):
            BS[:, l * NCH + m] = bias[l, m * 128:(m + 1) * 128]

    ON8 = np.ones((E, 1), np.float32)
    ON1 = np.ones((1, E), np.float32)
    shared = {"VT": VT, "UT": UT, "CB": CB, "GT": GT, "SEL": SEL, "BS": BS,
              "ON8": ON8, "ON1": ON1}
    for k in ("VT", "UT", "CB", "GT", "SEL", "ON8", "ON1"):
        shared[k] = shared[k].astype(ml_dtypes.bfloat16)
    xT = xT.astype(ml_dtypes.bfloat16)
    in_maps = []
    for i in range(NCORES):
        m = dict(shared)
        m["xT"] = np.ascontiguousarray(xT[:, i * BC:(i + 1) * BC])
        in_maps.append(m)
    return in_maps


def run(nc, in_maps):
    res = run_bass_kernel_spmd(nc, in_maps, core_ids=list(range(NCORES)))
    yT = np.empty((N, B), np.float32)
    for i in range(NCORES):
        yT[:, i * BC:(i + 1) * BC] = res.results[i]["y"]
    return np.ascontiguousarray(yT.T)


_NC_CACHE = {}


def kernel(x, U, V, C, bias, gate_w):
    x = np.asarray(x)
    assert x.shape == (B, N), f"expected x {(B, N)}, got {x.shape}"
    if "nc" not in _NC_CACHE:
        _NC_CACHE["nc"] = build(niter=1)
    in_maps = pack_inputs(x, U, V, C, bias, gate_w)
    return run(_NC_CACHE["nc"], in_maps)


# revision 15
# speedup vs baseline: 1.4621x; 1.0506x over previous
"""Bass/Tile TRN2 kernel for LowRankMixtureCrossNet (B=16384, N=1024, L=3, E=8, R=64).

Strategy:
- Data-parallel: batch sharded 8 ways (2048 tokens/core), weights replicated.
- On-chip layout is feature-major (x^T): SBUF tiles [128 feat, T=512 tokens].
  Host pre-transposes x and pre-packs the weights.
- All matmul operands in bf16 (1 cycle/column PE streaming; measured
  233 ns/MM for back-to-back 512-col bf16 matmuls incl. weight loads).
  PSUM accumulation stays fp32. Simulated numerics: 7.5e-3 max-rel-err
  (gate 2e-2); measured on HW 6.4e-3. fp8 variants simulate at 2.2-3e-2
  and fp8-weight matmuls measured SLOWER (313 ns/MM mixed-dtype penalty),
  so bf16 everywhere is both the precision and the speed floor.
- TWO token tiles (A, B) are processed as an interleaved wavefront per
  layer: A's serial softmax/elemwise chain (ACT/DVE) runs while B's
  matmul bursts occupy the PE, and vice versa. The residual updates
  (t2/xcur) are emitted AFTER both tiles' u-projection bursts so the DVE
  queue never blocks the other tile's in-chain ops.
- Per layer, per token tile:
    logits[e,t]  = sum_n gate_w[e,n] x[n,t]           (8 chunk matmuls, M=8)
    gates        = softmax over e: exp (ACT), partition sum + broadcast via
                   tiny PE matmuls against ones vectors, reciprocal+mult (DVE)
    v            = 4 expert-pair matmuls x 8 K-chunks, M=128 (2 experts x R=64)
    rvg          = relu(v) * gates  (gates folded in early:
                   g*U@relu(C@(g*relu(v))) == g*u since g>0 commutes via relu)
    cg           = relu(Cblk @ rvg)        (block-diag 2-expert C matmuls)
    w            = Uall^T.T @ cg           (8 n-chunks x 4 K-pair matmuls)
    xnew[n,t]    = (w[n,t] + bias[n]) * x0[n,t] + x[n,t]
      (softmax makes sum_e g = 1, so bias needs no gate weighting)
  The final layer's xnew is written fp32 and DMA'd out on the Pool queue.
"""
import numpy as np
from contextlib import ExitStack

import concourse.bass as bass
import concourse.tile as tile
from concourse import bacc, mybir
from concourse.bass_utils import run_bass_kernel_spmd

B, N, L, E, R = 16384, 1024, 3, 8, 64
NCORES = 8
BC = B // NCORES      # tokens per core
T = 512               # token tile (matmul free dim)
NT = BC // T          # token tiles per core
NPAIR = NT // 2       # interleaved tile pairs
NCH = N // 128        # feature chunks
NP = E // 2           # expert pairs
ER = E * R            # 512

f32 = mybir.dt.float32
bf16 = mybir.dt.bfloat16
AFT = mybir.ActivationFunctionType
ALU = mybir.AluOpType


def build(niter: int = 1, dma_in_loop=True, elemwise=True, pool_xcur=True,
          psum=(2, 2, 2, 1), hp=False, stagger=True):
    MDT = bf16
    nc = bacc.Bacc(trn_type="TRN2", debug=False, num_devices=NCORES)

    xT_d = nc.dram_tensor("xT", [N, BC], MDT, kind="ExternalInput")
    vt_d = nc.dram_tensor("VT", [L, N, ER], MDT, kind="ExternalInput")
    ut_d = nc.dram_tensor("UT", [L, ER, N], MDT, kind="ExternalInput")
    cb_d = nc.dram_tensor("CB", [L, 128, NP * 128], MDT, kind="ExternalInput")
    gt_d = nc.dram_tensor("GT", [N, E], MDT, kind="ExternalInput")
    sel_d = nc.dram_tensor("SEL", [E, NP * 128], MDT, kind="ExternalInput")
    bs_d = nc.dram_tensor("BS", [128, L * NCH], f32, kind="ExternalInput")
    on8_d = nc.dram_tensor("ON8", [E, 1], MDT, kind="ExternalInput")
    on1_d = nc.dram_tensor("ON1", [1, E], MDT, kind="ExternalInput")
    y_d = nc.dram_tensor("y", [N, BC], f32, kind="ExternalOutput")

    with tile.TileContext(nc) as tc, ExitStack() as ctx:
        wp = ctx.enter_context(tc.tile_pool(name="wp", bufs=1))
        xp = ctx.enter_context(tc.tile_pool(name="xp", bufs=2))
        xc = ctx.enter_context(tc.tile_pool(name="xc", bufs=2))
        yp = ctx.enter_context(tc.tile_pool(name="yp", bufs=1))
        wk = ctx.enter_context(tc.tile_pool(name="wk", bufs=3))
        g8 = ctx.enter_context(tc.tile_pool(name="g8", bufs=1))
        pv = ctx.enter_context(tc.tile_pool(name="pv", bufs=psum[0], space="PSUM"))
        pcg = ctx.enter_context(tc.tile_pool(name="pcg", bufs=psum[1], space="PSUM"))
        pw = ctx.enter_context(tc.tile_pool(name="pw", bufs=psum[2], space="PSUM"))
        psm = ctx.enter_context(tc.tile_pool(name="psm", bufs=psum[3], space="PSUM"))

        # ---- persistent weights ----
        vt, ut, cbt, gt = {}, {}, {}, {}

        def load_layer_weights(l, eng):
            tv = wp.tile([128, NCH * ER], MDT, tag=f"vt{l}", name=f"vt{l}")
            eng.dma_start(tv[:].rearrange("p (c e) -> p c e", c=NCH),
                          vt_d[l].rearrange("(c p) e -> p c e", p=128))
            for c in range(NCH):
                vt[l, c] = tv[:, c * ER:(c + 1) * ER]
            tu = wp.tile([128, NP * N], MDT, tag=f"ut{l}", name=f"ut{l}")
            eng.dma_start(tu[:].rearrange("p (k n) -> p k n", k=NP),
                          ut_d[l].rearrange("(k p) n -> p k n", p=128))
            for k in range(NP):
                ut[l, k] = tu[:, k * N:(k + 1) * N]
            t = wp.tile([128, NP * 128], MDT, tag=f"cb{l}", name=f"cb{l}")
            eng.dma_start(t[:], cb_d[l, :, :])
            cbt[l] = t

        # tiny operands + layer-0 V on the sync queue (critical path to the
        # first matmuls); the bulk (U0 + layers 1-2) on the scalar queue,
        # which is idle during preload.
        gtt = wp.tile([128, NCH * E], MDT, tag="gt", name="gtt")
        nc.sync.dma_start(gtt[:].rearrange("p (c e) -> p c e", c=NCH),
                          gt_d[:, :].rearrange("(c p) e -> p c e", p=128))
        for c in range(NCH):
            gt[c] = gtt[:, c * E:(c + 1) * E]
        selt = wp.tile([E, NP * 128], MDT, tag="sel", name="selt")
        nc.sync.dma_start(selt[:], sel_d[:, :])
        bst = wp.tile([128, L * NCH], f32, tag="bs", name="bst")
        nc.sync.dma_start(bst[:], bs_d[:, :])
        on8 = wp.tile([E, 1], MDT, tag="on8", name="on8")
        nc.sync.dma_start(on8[:], on8_d[:, :])
        on1 = wp.tile([1, E], MDT, tag="on1", name="on1")
        nc.sync.dma_start(on1[:], on1_d[:, :])
        tv = wp.tile([128, NCH * ER], MDT, tag="vt0", name="vt0")
        nc.sync.dma_start(tv[:].rearrange("p (c e) -> p c e", c=NCH),
                          vt_d[0].rearrange("(c p) e -> p c e", p=128))
        for c in range(NCH):
            vt[0, c] = tv[:, c * ER:(c + 1) * ER]
        tu = wp.tile([128, NP * N], MDT, tag="ut0", name="ut0")
        nc.scalar.dma_start(tu[:].rearrange("p (k n) -> p k n", k=NP),
                            ut_d[0].rearrange("(k p) n -> p k n", p=128))
        for k in range(NP):
            ut[0, k] = tu[:, k * N:(k + 1) * N]
        t0cb = wp.tile([128, NP * 128], MDT, tag="cb0", name="cb0")
        nc.scalar.dma_start(t0cb[:], cb_d[0, :, :])
        cbt[0] = t0cb
        for l in range(1, L):
            load_layer_weights(l, nc.scalar)

        uid = [0]
        x0_shared = {}

        class TileState:
            pass

        def start_tile(t, s):
            """Allocate per-tile wide tiles and start the x0 DMA (one wide
            transfer; chunk c occupies columns [c*T, (c+1)*T))."""
            u = uid[0]
            uid[0] += 1
            s.u = u
            s.t = t
            s.ab = "ab"[t % 2]
            ab = s.ab
            if dma_in_loop or t not in x0_shared:
                x0w = xp.tile([128, NCH * T], MDT, tag=f"x0{ab}", name=f"x0_{u}")
                HC = NCH // 2
                for h in range(2):
                    nc.sync.dma_start(
                        x0w[:, h * HC * T:(h + 1) * HC * T].rearrange(
                            "p (c t) -> p c t", c=HC),
                        xT_d.rearrange("(c p) b -> p c b", p=128)[
                            :, h * HC:(h + 1) * HC, t * T:(t + 1) * T])
                s.x0w = x0w
                s.x0 = [x0w[:, c * T:(c + 1) * T] for c in range(NCH)]
                if not dma_in_loop:
                    x0_shared[t] = (s.x0w, s.x0)
            else:
                s.x0w, s.x0 = x0_shared[t]
            xcw = xc.tile([128, NCH * T], MDT, tag=f"xc{ab}", name=f"xc_{u}")
            s.xcw = xcw
            s.xcur = [xcw[:, c * T:(c + 1) * T] for c in range(NCH)]
            s.ytw = yp.tile([128, NCH * T], f32, tag=f"y{ab}", name=f"y_{u}")
            s.yt = [s.ytw[:, c * T:(c + 1) * T] for c in range(NCH)]

        def emit_burst(s, l):
            """PE burst for one tile: gate logits + v matmuls with the
            softmax's tiny PE ops (S, r8, g2) threaded between v p-groups so
            the gate broadcast is ready by burst end without stalling PE.
            ACT: exp + relus trail; DVE: recip, gn, rvg trail."""
            u, ab = s.u, s.ab
            xin = s.x0 if l == 0 else s.xcur
            s.xin = xin
            s.xinw = s.x0w if l == 0 else s.xcw
            lg = psm.tile([E, T], f32, tag=f"lg{ab}", name=f"lg_{u}_{l}")
            for c in range(NCH):
                nc.tensor.matmul(lg[:], lhsT=gt[c][:], rhs=xin[c],
                                 start=(c == 0), stop=(c == NCH - 1))
            if elemwise:
                eh = g8.tile([E, T], MDT, tag=f"eh{ab}", name=f"eh_{u}_{l}")
                nc.scalar.activation(eh[:], lg[:], AFT.Exp)
                s.eh = eh
            s.rvs = {}
            g2s = {}

            def vgroup(p):
                vp = pv.tile([128, T], f32, tag="v", name=f"v_{u}_{l}_{p}")
                for c in range(NCH):
                    nc.tensor.matmul(vp[:], lhsT=vt[l, c][:, p * 128:(p + 1) * 128],
                                     rhs=xin[c],
                                     start=(c == 0), stop=(c == NCH - 1))
                if elemwise:
                    rv = wk.tile([128, T], MDT, tag=f"rv{ab}", name=f"rv_{u}_{l}_{p}",
                                 bufs=4)
                    nc.scalar.activation(rv[:], vp[:], AFT.Relu)
                    s.rvs[p] = rv

            vgroup(0)
            if elemwise:
                S = psm.tile([1, T], f32, tag=f"lg{ab}", name=f"S_{u}_{l}")
                nc.tensor.matmul(S[:], lhsT=on8[:], rhs=s.eh[:], start=True, stop=True)
                r1 = g8.tile([1, T], MDT, tag=f"r1{ab}", name=f"r1_{u}_{l}")
                with nc.allow_low_precision(reason="softmax recip to bf16"):
                    nc.vector.reciprocal(r1[:], S[:])
            vgroup(1)
            if elemwise:
                r8 = psm.tile([E, T], f32, tag=f"lg{ab}", name=f"r8_{u}_{l}")
                nc.tensor.matmul(r8[:], lhsT=on1[:], rhs=r1[:], start=True, stop=True)
                gn = g8.tile([E, T], MDT, tag=f"gn{ab}", name=f"gn_{u}_{l}")
                nc.vector.tensor_tensor(gn[:], s.eh[:], r8[:], op=ALU.mult)
            else:
                gn = selt
            vgroup(2)
            for p in range(NP):
                g2 = pcg.tile([128, T], f32, tag="cg2", name=f"g2_{u}_{l}_{p}")
                nc.tensor.matmul(g2[:], lhsT=selt[:, p * 128:(p + 1) * 128],
                                 rhs=gn[:, 0:T], start=True, stop=True)
                g2s[p] = g2
            s.rvgs = {}

            def rvgroup(p):
                if elemwise:
                    rvg = wk.tile([128, T], MDT, tag=f"rvg{ab}", name=f"rvg_{u}_{l}_{p}",
                                  bufs=4)
                    nc.vector.tensor_tensor(rvg[:], s.rvs[p][:], g2s[p][:], op=ALU.mult)
                    s.rvgs[p] = rvg[:]
                else:
                    s.rvgs[p] = s.x0[p]

            rvgroup(0)
            rvgroup(1)
            rvgroup(2)
            vgroup(3)
            rvgroup(3)

        def emit_cu(s, l):
            """PE: C matmuls + u-projection; ACT: cg relus trail."""
            u, ab = s.u, s.ab
            cg = {}
            for p in range(NP):
                cp = pcg.tile([128, T], f32, tag="cg2", name=f"c_{u}_{l}_{p}")
                nc.tensor.matmul(cp[:], lhsT=cbt[l][:, p * 128:(p + 1) * 128],
                                 rhs=s.rvgs[p], start=True, stop=True)
                if elemwise:
                    cgp = wk.tile([128, T], MDT, tag=f"cg{ab}{p}", name=f"cg_{u}_{l}_{p}",
                                  bufs=1)
                    nc.scalar.activation(cgp[:], cp[:], AFT.Relu)
                    cg[p] = cgp[:]
                else:
                    cg[p] = s.x0[p]
            s.wms = []
            for m in range(NCH):
                wm = pw.tile([128, T], f32, tag="w", name=f"w_{u}_{l}_{m}")
                for k in range(NP):
                    nc.tensor.matmul(wm[:], lhsT=ut[l, k][:, m * 128:(m + 1) * 128],
                                     rhs=cg[k],
                                     start=(k == 0), stop=(k == NP - 1))
                s.wms.append(wm)

        def emit_resid(s, l):
            """Residual update: xnew = (w + bias) * x0 + xin. t2 slices are
            written per-chunk (PSUM-bound), then the residual add runs as two
            half-wide ops; the last layer's adds (off the critical chain) go
            to the Pool engine and the output leaves as one wide DMA."""
            u, ab = s.u, s.ab
            last = l == L - 1
            outw = s.ytw if last else s.xcw
            if not elemwise:
                nc.vector.tensor_copy(outw[:], s.x0w[:])
            else:
                t2w = wk.tile([128, NCH * T], MDT, tag=f"t2{ab}", name=f"t2_{u}_{l}",
                              bufs=1)
                ops = []
                for m in range(NCH):
                    ops.append(lambda m=m: nc.vector.scalar_tensor_tensor(
                        t2w[:, m * T:(m + 1) * T], s.wms[m][:],
                        bst[:, l * NCH + m:l * NCH + m + 1],
                        s.x0[m], op0=ALU.add, op1=ALU.mult))
                if pool_xcur and last:
                    for m in range(NCH):
                        ops.append(lambda m=m: nc.gpsimd.tensor_tensor(
                            outw[:, m * T:(m + 1) * T], t2w[:, m * T:(m + 1) * T],
                            s.xinw[:, m * T:(m + 1) * T], op=ALU.add))
                else:
                    H = NCH * T // 2
                    for h in range(2):
                        ops.append(lambda h=h: nc.vector.tensor_tensor(
                            outw[:, h * H:(h + 1) * H], t2w[:, h * H:(h + 1) * H],
                            s.xinw[:, h * H:(h + 1) * H], op=ALU.add))
                if hp:
                    with tc.high_priority():
                        for op in ops:
                            op()
                else:
                    for op in ops:
                        op()
            if last and (dma_in_loop or s.t < 2):
                nc.gpsimd.dma_start(
                    y_d.rearrange("(c p) b -> p c b", p=128)[:, :, s.t * T:(s.t + 1) * T],
                    s.ytw[:].rearrange("p (c t) -> p c t", c=NCH))

        def pair_wave(tp):
            A, Bs = TileState(), TileState()
            start_tile(2 * tp, A)
            start_tile(2 * tp + 1, Bs)
            for l in range(L):
                emit_burst(A, l)
                emit_burst(Bs, l)
                emit_cu(A, l)
                emit_cu(Bs, l)
                emit_resid(A, l)
                emit_resid(Bs, l)

        if niter == 1:
            for tp in range(NPAIR):
                pair_wave(tp)
        else:
            with tc.For_i(0, niter, 1, staggered_reset=stagger) as _:
                for tp in range(NPAIR):
                    pair_wave(tp)

    nc.compile()
    return nc


def pack_inputs(x, U, V, C, bias, gate_w):
    """Host-side packing into the DRAM layouts the kernel expects."""
    import ml_dtypes
    x = np.asarray(x, dtype=np.float32)
    U = np.asarray(U, dtype=np.float32)
    V = np.asarray(V, dtype=np.float32)
    C = np.asarray(C, dtype=np.float32)
    bias = np.asarray(bias, dtype=np.float32)
    gate_w = np.asarray(gate_w, dtype=np.float32)

    xT = np.ascontiguousarray(x.T)                          # [N, B]
    VT = np.ascontiguousarray(V.transpose(0, 3, 1, 2).reshape(L, N, ER))
    UT = np.ascontiguousarray(U.transpose(0, 1, 3, 2).reshape(L, ER, N))
    CB = np.zeros((L, 128, NP * 128), np.float32)
    for l in range(L):
        for p in range(NP):
            CB[l, 0:64, p * 128:p * 128 + 64] = C[l, 2 * p].T
            CB[l, 64:128, p * 128 + 64:p * 128 + 128] = C[l, 2 * p + 1].T
    GT = np.ascontiguousarray(gate_w.T)                     # [N, E]
    SEL = np.zeros((E, NP * 128), np.float32)
    for p in range(NP):
        SEL[2 * p, p * 128:p * 128 + 64] = 1.0
        SEL[2 * p + 1, p * 128 + 64:p * 128 + 128] = 1.0
    BS = np.zeros((128, L * NCH), np.float32)
    for l in range(L):
        for m in range(NCH):
            BS[:, l * NCH + m] = bias[l, m * 128:(m + 1) * 128]

    ON8 = np.ones((E, 1), np.float32)
    ON1 = np.ones((1, E), np.float32)
    shared = {"VT": VT, "UT": UT, "CB": CB, "GT": GT, "SEL": SEL, "BS": BS,
              "ON8": ON8, "ON1": ON1}
    for k in ("VT", "UT", "CB", "GT", "SEL", "ON8", "ON1"):
        shared[k] = shared[k].astype(ml_dtypes.bfloat16)
    xT = xT.astype(ml_dtypes.bfloat16)
    in_maps = []
    for i in range(NCORES):
        m = dict(shared)
        m["xT"] = np.ascontiguousarray(xT[:, i * BC:(i + 1) * BC])
        in_maps.append(m)
    return in_maps


def run(nc, in_maps):
    res = run_bass_kernel_spmd(nc, in_maps, core_ids=list(range(NCORES)))
    yT = np.empty((N, B), np.float32)
    for i in range(NCORES):
        yT[:, i * BC:(i + 1) * BC] = res.results[i]["y"]
    return np.ascontiguousarray(yT.T)


_NC_CACHE = {}


def kernel(x, U, V, C, bias, gate_w):
    x = np.asarray(x)
    assert x.shape == (B, N), f"expected x {(B, N)}, got {x.shape}"
    if "nc" not in _NC_CACHE:
        _NC_CACHE["nc"] = build(niter=1)
    in_maps = pack_inputs(x, U, V, C, bias, gate_w)
    return run(_NC_CACHE["nc"], in_maps)
